# revision 1
# baseline (speedup 1.0000x reference)
"""ANOVA kernel (order 3) for Trainium2, 8 NeuronCores, pure data parallel.

Reference computation per sample b (x: (B, F, D) fp32):
    out[b] = sum_d e3(x[b, :, d])
where e3 is the 3rd elementary symmetric polynomial over the F=40 field values.

Newton's identities replace the sequential DP over F with power sums:
    p_k[b, d] = sum_f x[b, f, d]^k          (k = 1, 2, 3)
    e3 = (p1^3 - 3*p1*p2 + 2*p3) / 6
so the kernel is a pure streaming reduce — ideal for the memory-bound regime.

Per-core mapping (batch on partitions, so DMA is perfectly contiguous):
  - DMA x in f-chunks (region-aware deps start downstream work per chunk).
  - ScalarE: X2 = Square(X) (bf16 — plenty for p2 given the 2e-2 gate).
  - VectorE: X3 = X * X2 (bf16).
  - TensorE: p1/p2/p3 = sum_f via 40 PSUM-accumulating matmuls each, with an
    identity stationary matrix (fp32r moving for p1 at 1 cycle/row; bf16 for
    p2/p3). Redundant identity reloads are stripped post-schedule.
  - ScalarE/VectorE: e3 combination with scales folded into the PSUM->SBUF
    copies, reduce over d, per-tile output DMA.
"""

import numpy as np
from contextlib import ExitStack

import concourse.bacc as bacc
import concourse.mybir as mybir
import concourse.tile as tile
from concourse import masks
from concourse.bass_utils import run_bass_kernel_spmd
from bass_rust import add_dep_helper as bass_add_dep

N_CORES = 8
B, F, D = 16384, 40, 64
B_SHARD = B // N_CORES          # 2048 batches per core
NT = 4                          # sub-batches of 128 per tile
TILE_B = 128 * NT               # 512 batches per tile
NTILES = B_SHARD // TILE_B      # 4 tiles per core

FP32 = mybir.dt.float32
FP32R = mybir.dt.float32r
BF16 = mybir.dt.bfloat16
C_P1 = 6.0 ** (-1.0 / 3.0)
FC = 8                          # f-chunks per tile for DMA/sq/mul pipelining
FCH = F // FC
TAIL_SLOT = 7                   # X3 chunk index where prev tile's e3 drains


def _dedupe_ldweights(nc):
    """Remove InstLdweights that reload the weights already resident in the PE
    array (same stationary AP, no intervening self-loading fp32/fp32r matmul).
    Waits/updates of a removed load migrate to the next PE instruction."""
    PE = mybir.EngineType.PE
    removed = 0
    for block in nc.m.functions[0].blocks:
        insts = block.instructions
        cur_sig = None
        pending_sync = []
        keep = []
        for inst in insts:
            nm = type(inst).__name__
            if pending_sync and getattr(inst, "engine", None) == PE:
                si = inst.sync_info
                if si is None:
                    si = mybir.SyncInfo(on_wait=[], on_update=[])
                    inst.sync_info = si
                for psi in pending_sync:
                    si.on_wait = list(psi.on_wait) + list(si.on_wait)
                    si.on_update = list(si.on_update) + list(psi.on_update)
                pending_sync = []
            if nm == "InstMatmult":
                wap = inst.ins[1]
                if str(wap.dtype) in ("dt.float32", "dt.float32r",
                                      "float32", "float32r"):
                    cur_sig = None  # self-loading matmul clobbers the array
            elif nm == "InstLdweights":
                wap = inst.ins[0]
                sig = (str(wap.memref), wap.offset, str(wap.ap), str(wap.dtype))
                if sig == cur_sig:
                    si = inst.sync_info
                    if si is not None and (si.on_wait or si.on_update):
                        pending_sync.append(si)
                    removed += 1
                    continue
                cur_sig = sig
            keep.append(inst)
        assert not pending_sync, "dangling sync from removed trailing ldweights"
        block.instructions = keep
    return removed


def build_nc():
    nc = bacc.Bacc("TRN2", target_bir_lowering=False, debug=False,
                   num_devices=N_CORES)
    x = nc.dram_tensor("x", [B_SHARD, F, D], FP32, kind="ExternalInput")
    out = nc.dram_tensor("out", [B_SHARD, 1], FP32, kind="ExternalOutput")

    x_r = x.rearrange("(g p) f d -> p g f d", p=128)
    out_r = out.rearrange("(g p) o -> p (g o)", p=128)

    with tile.TileContext(nc) as tc, ExitStack() as ctx:
        const = ctx.enter_context(tc.tile_pool(name="const", bufs=1))
        xp = ctx.enter_context(tc.tile_pool(name="xp", bufs=2))
        x2p = ctx.enter_context(tc.tile_pool(name="x2p", bufs=2))
        x3p = ctx.enter_context(tc.tile_pool(name="x3p", bufs=2))
        pp = ctx.enter_context(tc.tile_pool(name="pp", bufs=2))
        tp = ctx.enter_context(tc.tile_pool(name="tp", bufs=2))
        small = ctx.enter_context(tc.tile_pool(name="small", bufs=2))
        psum = ctx.enter_context(tc.tile_pool(name="psum", bufs=2, space="PSUM"))

        ident_bf16 = const.tile([128, 128], BF16)
        masks.make_identity(nc, ident_bf16[:])
        # fp32r identity: values 0/1 are exact; a DVE copy gives the verifier
        # the "rounded to fp32r" provenance the fp32r matmul inputs require.
        ident_f32r = const.tile([128, 128], FP32R)
        nc.vector.tensor_copy(ident_f32r[:], ident_bf16[:])
        # identity scaled by 1/3 for the p3 group: p3ps accumulates p3/3
        # directly, removing the separate DVE scale op from the tail.
        # (bf16(1/3) has ~2e-3 relative error on a term worth ~2% of the
        # output — far inside the 2e-2 gate.)
        ident_third = const.tile([128, 128], BF16)
        nc.vector.tensor_scalar_mul(ident_third[:], ident_bf16[:], 1.0 / 3.0)

        pending = {}

        def emit_tail(i):
            """PSUM -> SBUF + e3 combine for tile i (emitted one tile late so
            next tile's squares never queue behind these on the ACT engine)."""
            g0, nt, p1ps, p2ps, p3ps = pending.pop(i)
            # with c = 6^(-1/3):  e3 = (c*p1)^3 - (c*p1)*(3c^2*p2) + p3/3
            p1c = pp.tile([128, nt, D], FP32, tag="p1c")
            nc.scalar.mul(p1c[:], p1ps[:], C_P1)
            p2m = pp.tile([128, nt, D], FP32, tag="p2m")
            nc.scalar.mul(p2m[:], p2ps[:], -3.0 * C_P1 * C_P1)

            # e3 per (b, d), then reduce over d straight into the output slot
            t1 = tp.tile([128, nt, D], FP32, tag="t1")
            nc.vector.tensor_mul(t1[:], p1c[:], p1c[:])
            t3 = tp.tile([128, nt, D], FP32, tag="t3")
            nc.vector.tensor_add(t3[:], t1[:], p2m[:])
            t4 = tp.tile([128, nt, D], FP32, tag="t4")
            nc.vector.tensor_mul(t4[:], t3[:], p1c[:])
            # Split the d-reduce: sum(t4) runs before the p3 group stops;
            # only the p3ps reduce + a tiny add remain on the post-p3 chain.
            # (p3ps already holds p3/3 via the scaled identity.)
            r4 = small.tile([128, nt], FP32, tag="r4")
            nc.vector.reduce_sum(r4[:], t4[:], axis=mybir.AxisListType.X)
            r3 = small.tile([128, nt], FP32, tag="r3")
            nc.vector.reduce_sum(r3[:], p3ps[:], axis=mybir.AxisListType.X)
            ob = small.tile([128, nt], FP32, tag="ob")
            red = nc.vector.tensor_add(ob[:], r4[:], r3[:])
            nc.sync.dma_start(out_r[:, g0:g0 + nt], ob[:])
            return red

        # Tile sizes in 128-batch groups. Uniform 512-batch tiles measured
        # best: shrinking the trailing tiles cuts the exposed end chain but
        # costs more per-chunk/instruction overhead than it saves.
        tile_nts = [4] * NTILES
        assert sum(tile_nts) == B_SHARD // 128
        g0 = 0
        for i, nt in enumerate(tile_nts):
            last = i == len(tile_nts) - 1
            X = xp.tile([128, nt, F, D], FP32, tag="X")
            X2 = x2p.tile([128, nt, F, D], BF16, tag="X2")
            X3 = x3p.tile([128, nt, F, D], BF16, tag="X3")
            # f-chunked DMA/square/mul: region-aware deps let the per-fi
            # matmuls (and sq/mul) start as soon as each f-chunk lands, so
            # the pipeline fill is one chunk (~1/FC tile), not a whole tile.
            # The last tile's final chunks taper so the end-of-kernel
            # DMA->sq->mul->matmul chain runs on a small remainder.
            if last:
                sizes = [FCH] * (FC - 1) + [3, 1, 1]
            elif i == 0:
                sizes = [2, 3] + [FCH] * (FC - 1)
            else:
                sizes = [FCH] * FC
            assert sum(sizes) == F
            bounds = [0]
            for s in sizes:
                bounds.append(bounds[-1] + s)
            chunks = [slice(a, b) for a, b in zip(bounds[:-1], bounds[1:])]
            for fs in chunks:
                # fp32r-typed byte copy (same bits) so the fp32r matmul sees
                # a properly-typed producer; other engines read the fp32 view.
                nc.sync.dma_start(X[:, :, fs, :].bitcast(FP32R),
                                  x_r[:, g0:g0 + nt, fs, :].bitcast(FP32R))
            for fs in chunks:
                nc.scalar.square(X2[:, :, fs, :], X[:, :, fs, :])
            for ci, fs in enumerate(chunks):
                if ci == TAIL_SLOT and i >= 1:
                    # Drain the previous tile's e3 chain here rather than
                    # behind this tile's full X3 stream on the in-order DVE.
                    prev_red = emit_tail(i - 1)
                mul = nc.vector.tensor_mul(X3[:, :, fs, :], X[:, :, fs, :],
                                           X2[:, :, fs, :])
                if ci == TAIL_SLOT and i >= 1 and prev_red is not None:
                    # ordering-only edge: the greedy scheduler would otherwise
                    # queue every X3 chunk (ready early) ahead of the e3 ops
                    bass_add_dep(mul.ins, prev_red.ins, sync=False,
                                 reason="drain prev-tile e3 before late X3")

            # Power-sum accumulation groups. Each matmul spans two fi values
            # with a stride-0 (broadcast) output AP: the PSUM has_written
            # bits accumulate the same-address column writes, so one N=2*nt*D
            # matmul replaces two (fewer instructions and sem increments).
            p1ps = psum.tile([128, nt, D], FP32, tag="p1ps")
            p2ps = psum.tile([128, nt, D], FP32, tag="p2ps")
            p3ps = psum.tile([128, nt, D], FP32, tag="p3ps")
            Xr = X[:].bitcast(FP32R)

            def bcast(ps, r):
                ap = ps[:]
                return ap.__replace__(ap=[ap.ap[0], ap.ap[1], [0, r], ap.ap[2]])

            def mm_group(ps, ident, src, r):
                for f0 in range(0, F, r):
                    nc.tensor.matmul(bcast(ps, r), lhsT=ident,
                                     rhs=src[:, :, f0:f0 + r, :],
                                     start=(f0 == 0), stop=(f0 + r >= F),
                                     skip_group_check=True)

            mm_group(p1ps, ident_f32r[:], Xr, 2)    # N<=512 (ISA max per mm)
            mm_group(p2ps, ident_bf16[:], X2, 2)
            mm_group(p3ps, ident_third[:], X3, 2)

            pending[i] = (g0, nt, p1ps, p2ps, p3ps)
            g0 += nt
        emit_tail(len(tile_nts) - 1)

    _dedupe_ldweights(nc)
    nc.finalize()
    return nc


_NC_CACHE = None


def _get_nc():
    global _NC_CACHE
    if _NC_CACHE is None:
        _NC_CACHE = build_nc()
    return _NC_CACHE


def run(x: np.ndarray, **spmd_kwargs):
    """Run on 8 cores; returns (out (B,1) fp32, BassKernelResults)."""
    assert x.shape == (B, F, D), x.shape
    x = np.ascontiguousarray(x, dtype=np.float32)
    nc = _get_nc()
    in_maps = [{"x": x[i * B_SHARD:(i + 1) * B_SHARD]} for i in range(N_CORES)]
    res = run_bass_kernel_spmd(nc, in_maps, core_ids=list(range(N_CORES)),
                               **spmd_kwargs)
    out = np.concatenate([res.results[i]["out"] for i in range(N_CORES)], axis=0)
    return out, res


def kernel(x: np.ndarray) -> np.ndarray:
    out, _ = run(x)
    return out


if __name__ == "__main__":
    rng = np.random.default_rng(0)
    x = rng.standard_normal((B, F, D)).astype(np.float32)
    out = kernel(x)
    print("out", out.shape, out.dtype, out[:4, 0])



# revision 4
# speedup vs baseline: 1.1253x; 1.1253x over previous
"""ANOVA kernel (order 3) for Trainium2, 8 NeuronCores, pure data parallel.

Reference computation per sample b (x: (B, F, D) fp32):
    out[b] = sum_d e3(x[b, :, d])
where e3 is the 3rd elementary symmetric polynomial over the F=40 field values.

Newton's identities replace the sequential DP over F with power sums:
    p_k[b, d] = sum_f x[b, f, d]^k          (k = 1, 2, 3)
    e3 = (p1^3 - 3*p1*p2 + 2*p3) / 6

Per-core mapping (batch on partitions, 16 groups of 128 batches):
  - gpsimd (SWDGE) casting DMA streams x fp32->fp16 in f-chunks. The DMA
    transfer cost scales with the *output* bytes, so casting halves the
    HBM stream time; fp16 keeps ~5e-4 relative precision (gate is 2e-2).
  - ScalarE: X2 = Square(X) (fp16).
  - VectorE: X3 = X * X2 (fp16).
  - TensorE: p1/p2/p3 = sum_f via PSUM-accumulating matmuls with scaled
    identity stationaries (scales folded: p1ps = c*p1 with c = 6^(-1/3),
    p2ps = -3c^2*p2, p3ps = p3/3 accumulated over BOTH f and d via a
    stride-0 output AP).
  - Tail per tile: t1 = p1ps^2 (ACT), t3 = t1 + p2ps, t4 = t3 * p1ps
    (DVE), then one matmul accumulates sum_d t4 into p3ps so PSUM holds
    the final per-group outputs; one copy into an SBUF staging tile.
  - One output DMA of the fp32 [128, 16] staging tile at the end
    (per-partition contiguous, so it costs 56 ns instead of 224/group).
"""

import numpy as np
from contextlib import ExitStack

import concourse.bacc as bacc
import concourse.mybir as mybir
import concourse.tile as tile
from concourse import masks
from concourse.bass_utils import run_bass_kernel_spmd
from bass_rust import add_dep_helper as bass_add_dep

N_CORES = 8
B, F, D = 16384, 40, 64
B_SHARD = B // N_CORES          # 2048 batches per core
GROUPS = B_SHARD // 128         # 16 groups of 128 batches

FP32 = mybir.dt.float32
FP16 = mybir.dt.float16
C1 = 6.0 ** (-1.0 / 3.0)        # p1 scale (folded into identity)
C2 = -3.0 * 6.0 ** (-2.0 / 3.0)  # p2 scale
C3 = 1.0 / 3.0                   # p3 scale

# tile sizes in 128-batch groups and f-chunk splits per tile
TILE_NTS = [4, 4, 4, 4]
TAIL_SLOT = 2                   # chunk index where prev tile's tail drains


def _dedupe_ldweights(nc):
    """Remove InstLdweights that reload weights already resident in the PE
    array. Waits/updates of a removed load migrate to the next PE inst."""
    PE = mybir.EngineType.PE
    removed = 0
    for block in nc.m.functions[0].blocks:
        insts = block.instructions
        cur_sig = None
        pending_sync = []
        keep = []
        for inst in insts:
            nm = type(inst).__name__
            if pending_sync and getattr(inst, "engine", None) == PE:
                si = inst.sync_info
                if si is None:
                    si = mybir.SyncInfo(on_wait=[], on_update=[])
                    inst.sync_info = si
                for psi in pending_sync:
                    si.on_wait = list(psi.on_wait) + list(si.on_wait)
                    si.on_update = list(si.on_update) + list(psi.on_update)
                pending_sync = []
            if nm == "InstMatmult":
                wap = inst.ins[1]
                if str(wap.dtype) in ("dt.float32", "dt.float32r",
                                      "float32", "float32r"):
                    cur_sig = None  # self-loading matmul clobbers the array
            elif nm == "InstLdweights":
                wap = inst.ins[0]
                sig = (str(wap.memref), wap.offset, str(wap.ap), str(wap.dtype))
                if sig == cur_sig:
                    si = inst.sync_info
                    if si is not None and (si.on_wait or si.on_update):
                        pending_sync.append(si)
                    removed += 1
                    continue
                cur_sig = sig
            keep.append(inst)
        assert not pending_sync, "dangling sync from removed trailing ldweights"
        block.instructions = keep
    return removed


def build_nc():
    nc = bacc.Bacc("TRN2", target_bir_lowering=False, debug=False,
                   num_devices=N_CORES, dynamic_dma_scratch_size=65536)
    x = nc.dram_tensor("x", [B_SHARD, F, D], FP32, kind="ExternalInput")
    out = nc.dram_tensor("out", [128, GROUPS], FP32, kind="ExternalOutput")

    x_r = x.rearrange("(g p) f d -> p g f d", p=128)

    with tile.TileContext(nc) as tc, ExitStack() as ctx:
        const = ctx.enter_context(tc.tile_pool(name="const", bufs=1))
        xp = ctx.enter_context(tc.tile_pool(name="xp", bufs=2))
        x2p = ctx.enter_context(tc.tile_pool(name="x2p", bufs=2))
        x3p = ctx.enter_context(tc.tile_pool(name="x3p", bufs=2))
        tp = ctx.enter_context(tc.tile_pool(name="tp", bufs=2))
        stg = ctx.enter_context(tc.tile_pool(name="stg", bufs=1))
        psum = ctx.enter_context(tc.tile_pool(name="psum", bufs=2, space="PSUM"))

        ident = const.tile([128, 128], FP16)
        masks.make_identity(nc, ident[:])
        ident_c1 = const.tile([128, 128], FP16)
        nc.vector.tensor_scalar_mul(ident_c1[:], ident[:], C1)
        ident_c2 = const.tile([128, 128], FP16)
        nc.vector.tensor_scalar_mul(ident_c2[:], ident[:], C2)
        ident_c3 = const.tile([128, 128], FP16)
        nc.vector.tensor_scalar_mul(ident_c3[:], ident[:], C3)

        stage = stg.tile([128, GROUPS], FP32, tag="stage")

        pending = {}

        def emit_tail(i):
            """Tail of tile i: e3 combine feeding sum_d into p3ps, then a
            PSUM->SBUF staging copy. Emitted one tile late so the next
            tile's stream work keeps the engines' queues primed."""
            g0, nt, p1ps, p2ps, p3ps = pending.pop(i)
            t1 = tp.tile([128, nt, D], FP16, tag="t1")
            nc.scalar.square(t1[:], p1ps[:])               # c^2 p1^2
            t3 = tp.tile([128, nt, D], FP16, tag="t3")
            nc.vector.tensor_add(t3[:], t1[:], p2ps[:])    # c^2(p1^2-3p2)
            t4 = tp.tile([128, nt, D], FP16, tag="t4")
            mul = nc.vector.tensor_mul(t4[:], t3[:], p1ps[:])
            # close the p3 PSUM group: p3ps += sum_d t4. The innermost out
            # dim must be a real (nonzero-stride) dim for same-address PSUM
            # accumulation, so move d outward and keep the group dim inner.
            ap = p3ps[:]
            bcast_d = ap.__replace__(ap=[ap.ap[0], [0, D], ap.ap[1]])
            nc.tensor.matmul(bcast_d, lhsT=ident[:],
                             rhs=t4[:].rearrange("p g d -> p d g"),
                             start=False, stop=True, skip_group_check=True)
            cp = nc.scalar.copy(stage[:, g0:g0 + nt], p3ps[:])
            return mul, cp

        g0 = 0
        for i, nt in enumerate(TILE_NTS):
            X = xp.tile([128, nt, F, D], FP16, tag="X")
            X2 = x2p.tile([128, nt, F, D], FP16, tag="X2")
            X3 = x3p.tile([128, nt, F, D], FP16, tag="X3")
            sizes = [10, 10, 10, 10]
            assert sum(sizes) == F
            bounds = [0]
            for s in sizes:
                bounds.append(bounds[-1] + s)
            chunks = [slice(a, b) for a, b in zip(bounds[:-1], bounds[1:])]
            # fp32 -> fp16 casting stream (SWDGE): cost scales with fp16 bytes
            for fs in chunks:
                nc.gpsimd.dma_start(X[:, :, fs, :], x_r[:, g0:g0 + nt, fs, :])
            for fs in chunks:
                nc.scalar.square(X2[:, :, fs, :], X[:, :, fs, :])
            prev_ops = None
            for ci, fs in enumerate(chunks):
                if ci == TAIL_SLOT and i >= 1:
                    prev_ops = emit_tail(i - 1)
                mul = nc.vector.tensor_mul(X3[:, :, fs, :], X[:, :, fs, :],
                                           X2[:, :, fs, :])
                if ci == TAIL_SLOT and prev_ops is not None:
                    # ordering-only edge: keep the prev tile's combine ahead
                    # of this tile's remaining X3 stream in the DVE queue
                    bass_add_dep(mul.ins, prev_ops[0].ins, sync=False,
                                 reason="drain prev-tile combine first")

            p1ps = psum.tile([128, nt, D], FP32, tag="p1ps")
            p2ps = psum.tile([128, nt, D], FP32, tag="p2ps")
            p3ps = psum.tile([128, nt], FP32, tag="p3ps")

            def bcast_r(ps, r):
                ap = ps[:]
                return ap.__replace__(ap=[ap.ap[0], ap.ap[1], [0, r], ap.ap[2]])

            def bcast_rd(ps, r):
                # stride-0 dims must not be innermost: order (r, d, g)
                ap = ps[:]
                return ap.__replace__(ap=[ap.ap[0], [0, r], [0, D], ap.ap[1]])

            R = 2

            def mm_group(out_ap_fn, ps, ident_ap, src, stop_last=True,
                         reorder=False):
                for f0 in range(0, F, R):
                    mv = src[:, :, f0:f0 + R, :]
                    if reorder:
                        mv = mv.rearrange("p g f d -> p f d g")
                    nc.tensor.matmul(out_ap_fn(ps, R), lhsT=ident_ap,
                                     rhs=mv,
                                     start=(f0 == 0),
                                     stop=(stop_last and f0 + R >= F),
                                     skip_group_check=True)

            mm_group(bcast_r, p1ps, ident_c1[:], X)
            mm_group(bcast_r, p2ps, ident_c2[:], X2)
            # p3 group left open: the tail's sum_d matmul closes it
            mm_group(bcast_rd, p3ps, ident_c3[:], X3, stop_last=False,
                     reorder=True)

            pending[i] = (g0, nt, p1ps, p2ps, p3ps)
            g0 += nt
        emit_tail(len(TILE_NTS) - 1)
        nc.sync.dma_start(out[:], stage[:])

    _dedupe_ldweights(nc)
    nc.finalize()
    return nc


_NC_CACHE = None


def _get_nc():
    global _NC_CACHE
    if _NC_CACHE is None:
        _NC_CACHE = build_nc()
    return _NC_CACHE


def run(x: np.ndarray, **spmd_kwargs):
    """Run on 8 cores; returns (out (B,1) fp32, BassKernelResults)."""
    assert x.shape == (B, F, D), x.shape
    x = np.ascontiguousarray(x, dtype=np.float32)
    nc = _get_nc()
    in_maps = [{"x": x[i * B_SHARD:(i + 1) * B_SHARD]} for i in range(N_CORES)]
    res = run_bass_kernel_spmd(nc, in_maps, core_ids=list(range(N_CORES)),
                               **spmd_kwargs)
    outs = []
    for i in range(N_CORES):
        o = res.results[i]["out"]          # [128, GROUPS], out[p, g] = b g*128+p
        outs.append(o.T.reshape(B_SHARD, 1))
    return np.concatenate(outs, axis=0), res


def kernel(x: np.ndarray) -> np.ndarray:
    out, _ = run(x)
    return out


if __name__ == "__main__":
    rng = np.random.default_rng(0)
    x = rng.standard_normal((B, F, D)).astype(np.float32)
    out = kernel(x)
    print("out", out.shape, out.dtype, out[:4, 0])
